# revision 4
# baseline (speedup 1.0000x reference)
"""Bass/Tile kernel for nn_AttnModule (sparse_attention).

Reference computation (per batch b):
    scores  = pos_emb @ position_fmap[b].T          # [T, L]
    attn    = softmax(scores, axis=-1)              # softmax over L
    context = attn @ origin_fmap[b]                 # [T, H]
    out     = context @ W_gen.T + b_gen             # [T, C]

Sharding: pure data parallel over batch B=64 -> 8 cores x 8 batches.

Host-side layout choices (free):
  - position_fmap shipped pre-transposed per batch: pfT [B, H, L]
    (H on partitions = contraction dim for scores matmul)
  - pos_emb shipped transposed: peT [H, T]
  - W_gen shipped transposed: wgT [H, C]
  - output produced as [B, C, T] and transposed back on host.
"""

import numpy as np

import concourse.bass as bass
import concourse.mybir as mybir
import concourse.tile as tile
from concourse import bacc
from concourse.bass_utils import run_bass_kernel_spmd
from concourse.masks import make_identity

B, L, H, T, C = 64, 1024, 512, 100, 97
NCORES = 8
BPC = B // NCORES  # batches per core

HT = H // 128  # 4 h-tiles
LT = L // 128  # 8 l-tiles

F32 = mybir.dt.float32
AF = mybir.ActivationFunctionType
AX = mybir.AxisListType
OP = mybir.AluOpType


def build_nc(mm_dt=F32, repeats=1):
    """Build the per-core Bass program. mm_dt = dtype of matmul inputs.

    repeats>1 re-runs the whole computation in-NEFF (for differential
    timing: per-iter = (t(R2)-t(R1))/(R2-R1) cancels launch overheads).
    """
    nc = bacc.Bacc(None, target_bir_lowering=False, debug=False)

    pfT = nc.dram_tensor("pfT", [BPC, H, L], mm_dt, kind="ExternalInput").ap()
    of = nc.dram_tensor("of", [BPC, L, H], mm_dt, kind="ExternalInput").ap()
    peT = nc.dram_tensor("peT", [H, T], mm_dt, kind="ExternalInput").ap()
    wgT = nc.dram_tensor("wgT", [H, C], mm_dt, kind="ExternalInput").ap()
    bg = nc.dram_tensor("bg", [C, 1], F32, kind="ExternalInput").ap()
    outT = nc.dram_tensor("outT", [BPC, C, T], F32, kind="ExternalOutput").ap()

    with tile.TileContext(nc) as tc:
        with (
            tc.tile_pool(name="consts", bufs=1) as consts,
            tc.tile_pool(name="pf", bufs=2) as pfpool,
            tc.tile_pool(name="ofp", bufs=2) as ofpool,
            tc.tile_pool(name="work", bufs=2) as work,
            tc.tile_pool(name="ps_scores", bufs=2, space="PSUM") as ps_scores,
            tc.tile_pool(name="ps_tp", bufs=2, space="PSUM") as ps_tp,
            tc.tile_pool(name="ps_ctx", bufs=1, space="PSUM") as ps_ctx,
            tc.tile_pool(name="ps_out", bufs=1, space="PSUM") as ps_out,
        ):
            # ---- constants (loaded once) ----
            peT_sb = consts.tile([128, HT, T], mm_dt)
            nc.sync.dma_start(peT_sb, peT.rearrange("(ht p) t -> p ht t", p=128))
            wgT_sb = consts.tile([128, HT, C], mm_dt)
            nc.sync.dma_start(wgT_sb, wgT.rearrange("(ht p) c -> p ht c", p=128))
            bg_sb = consts.tile([C, 1], F32)
            nc.sync.dma_start(bg_sb, bg)
            ident = consts.tile([128, 128], mm_dt)
            make_identity(nc, ident)

            for _rep in range(repeats):
              for b in range(BPC):
                # ---- load inputs for this batch ----
                pf_sb = pfpool.tile([128, HT, L], mm_dt, tag="pf")
                nc.sync.dma_start(pf_sb, pfT[b].rearrange("(ht p) l -> p ht l", p=128))
                of_sb = ofpool.tile([128, LT, H], mm_dt, tag="of")
                nc.sync.dma_start(of_sb, of[b].rearrange("(lt p) h -> p lt h", p=128))

                # ---- matmul 1: scores[T, L] += peT[h]^T @ pfT[h] ----
                sc_ps = ps_scores.tile([T, L], F32, tag="scores")
                for lh in range(L // 512):
                    for ht in range(HT):
                        nc.tensor.matmul(
                            sc_ps[:, lh * 512 : (lh + 1) * 512],
                            lhsT=peT_sb[:, ht, :],
                            rhs=pf_sb[:, ht, lh * 512 : (lh + 1) * 512],
                            start=(ht == 0),
                            stop=(ht == HT - 1),
                        )

                # ---- softmax over free dim (L) ----
                negm = work.tile([T, 1], F32, tag="negm")
                nc.vector.tensor_reduce(negm, sc_ps[:], axis=AX.X, op=OP.max, negate=True)
                p_sb = work.tile([T, L], mm_dt, tag="p")
                ssum = work.tile([T, 1], F32, tag="ssum")
                nc.scalar.activation(p_sb, sc_ps[:], AF.Exp, bias=negm, scale=1.0, accum_out=ssum)
                rinv = work.tile([T, 1], F32, tag="rinv")
                nc.vector.reciprocal(rinv, ssum)

                # ---- transpose P: [T, L] -> pT [128, LT, T] (PE transpose) ----
                pT_sb = work.tile([128, LT, T], mm_dt, tag="pT")
                for lt in range(LT):
                    tp_ps = ps_tp.tile([128, T], F32, tag="tp")
                    nc.tensor.transpose(tp_ps, p_sb[:, lt * 128 : (lt + 1) * 128], ident[:T, :T])
                    if lt % 2 == 0:
                        nc.vector.tensor_copy(pT_sb[:, lt, :], tp_ps)
                    else:
                        nc.scalar.copy(pT_sb[:, lt, :], tp_ps)

                # ---- matmul 2: ctx[T, H] += pT[l]^T @ of[l] ----
                ctx_ps = ps_ctx.tile([T, H], F32, tag="ctx")
                for lt in range(LT):
                    nc.tensor.matmul(
                        ctx_ps,
                        lhsT=pT_sb[:, lt, :],
                        rhs=of_sb[:, lt, :],
                        start=(lt == 0),
                        stop=(lt == LT - 1),
                    )
                # scale by 1/sum during PSUM->SBUF copy (+ cast)
                ctx_sb = work.tile([T, H], mm_dt, tag="ctx_sb")
                nc.vector.tensor_scalar_mul(ctx_sb, ctx_ps[:], rinv)

                # ---- transpose ctx: [T, H] -> cT [128, HT, T] ----
                cT_sb = work.tile([128, HT, T], mm_dt, tag="cT")
                for ht in range(HT):
                    tp_ps = ps_tp.tile([128, T], F32, tag="tp")
                    nc.tensor.transpose(tp_ps, ctx_sb[:, ht * 128 : (ht + 1) * 128], ident[:T, :T])
                    if ht % 2 == 0:
                        nc.vector.tensor_copy(cT_sb[:, ht, :], tp_ps)
                    else:
                        nc.scalar.copy(cT_sb[:, ht, :], tp_ps)

                # ---- matmul 3: outT[C, T] += wgT[h]^T @ cT[h] ----
                o_ps = ps_out.tile([C, T], F32, tag="out")
                for ht in range(HT):
                    nc.tensor.matmul(
                        o_ps,
                        lhsT=wgT_sb[:, ht, :],
                        rhs=cT_sb[:, ht, :],
                        start=(ht == 0),
                        stop=(ht == HT - 1),
                    )
                out_sb = work.tile([C, T], F32, tag="out_sb")
                nc.vector.tensor_scalar_add(out_sb, o_ps[:], bg_sb)
                nc.sync.dma_start(outT[b], out_sb)

    nc.compile()
    return nc


_NC = None


def _get_nc():
    global _NC
    if _NC is None:
        _NC = build_nc()
    return _NC


def make_in_maps(position_fmap, origin_fmap, pos_emb, W_gen, b_gen, np_dt=np.float32):
    """Host-side sharding + layout prep. Returns list of per-core input dicts."""
    pf = np.asarray(position_fmap, dtype=np.float32)
    of = np.asarray(origin_fmap, dtype=np.float32)
    pe = np.asarray(pos_emb, dtype=np.float32)
    wg = np.asarray(W_gen, dtype=np.float32)
    bgv = np.asarray(b_gen, dtype=np.float32)

    # [B, L, H] -> [B, H, L], contiguous
    pfT = np.ascontiguousarray(pf.transpose(0, 2, 1)).astype(np_dt)
    of_c = np.ascontiguousarray(of).astype(np_dt)
    peT = np.ascontiguousarray(pe.T).astype(np_dt)
    wgT = np.ascontiguousarray(wg.T).astype(np_dt)
    bg2 = np.ascontiguousarray(bgv.reshape(C, 1)).astype(np.float32)

    in_maps = []
    for i in range(NCORES):
        sl = slice(i * BPC, (i + 1) * BPC)
        in_maps.append(
            {
                "pfT": pfT[sl],
                "of": of_c[sl],
                "peT": peT,
                "wgT": wgT,
                "bg": bg2,
            }
        )
    return in_maps


def kernel(position_fmap, origin_fmap, pos_emb, W_gen, b_gen):
    nc = _get_nc()
    in_maps = make_in_maps(position_fmap, origin_fmap, pos_emb, W_gen, b_gen)
    res = run_bass_kernel_spmd(nc, in_maps, core_ids=list(range(NCORES)))
    outs = [r["outT"] for r in res.results]  # each [BPC, C, T]
    out = np.concatenate(outs, axis=0)  # [B, C, T]
    return np.ascontiguousarray(out.transpose(0, 2, 1)).astype(np.float32)


# revision 6
# speedup vs baseline: 2.4155x; 2.4155x over previous
"""Bass/Tile kernel for nn_AttnModule (sparse_attention).

Reference computation (per batch b):
    scores  = pos_emb @ position_fmap[b].T          # [T, L]
    attn    = softmax(scores, axis=-1)              # softmax over L
    context = attn @ origin_fmap[b]                 # [T, H]
    out     = context @ W_gen.T + b_gen             # [T, C]

Sharding: pure data parallel over batch B=64 -> 8 cores x 8 batches.

Dtype strategy: all matmuls in fp16 (1 cycle/row on PE, 2-byte DMA);
pos_emb is shipped as an fp16 hi/lo pair and mm1 accumulates both terms
in fp32 PSUM, which removes the dominant logit-rounding error (softmax
here is extremely peaked: scores ~ N(0, 512) unscaled). Softmax
statistics are fp32 throughout.

Host-side layout choices (free):
  - position_fmap shipped pre-transposed per batch: pfT [B, H, L]
    (H on partitions = contraction dim for scores matmul)
  - pos_emb shipped transposed as hi/lo fp16 pair: peT [2, H, T]
  - W_gen shipped transposed: wgT [H, C]
  - output produced as [B, C, T] and transposed back on host.
"""

import numpy as np

import concourse.bass as bass
import concourse.mybir as mybir
import concourse.tile as tile
from concourse import bacc
from concourse.bass_utils import run_bass_kernel_spmd
from concourse.masks import make_identity

B, L, H, T, C = 64, 1024, 512, 100, 97
NCORES = 8
BPC = B // NCORES  # batches per core

HT = H // 128  # 4 h-tiles
LT = L // 128  # 8 l-tiles

F32 = mybir.dt.float32
AF = mybir.ActivationFunctionType
AX = mybir.AxisListType
OP = mybir.AluOpType

# matmul input dtype config
MM_DT = mybir.dt.float16
NP_DT = np.float16
PE_TERMS = 2  # pos_emb hi/lo pair


def build_nc(mm_dt=MM_DT, pe_terms=PE_TERMS, repeats=1):
    """Build the per-core Bass program.

    repeats>1 re-runs the whole computation in-NEFF (for differential
    timing: per-iter = (t(R2)-t(R1))/(R2-R1) cancels launch overheads).
    """
    nc = bacc.Bacc(None, target_bir_lowering=False, debug=False)

    pfT = nc.dram_tensor("pfT", [BPC, H, L], mm_dt, kind="ExternalInput").ap()
    of = nc.dram_tensor("of", [BPC, L, H], mm_dt, kind="ExternalInput").ap()
    peT = nc.dram_tensor("peT", [pe_terms, H, T], mm_dt, kind="ExternalInput").ap()
    wgT = nc.dram_tensor("wgT", [H, C], mm_dt, kind="ExternalInput").ap()
    bg = nc.dram_tensor("bg", [C, 1], F32, kind="ExternalInput").ap()
    outT = nc.dram_tensor("outT", [BPC, C, T], F32, kind="ExternalOutput").ap()

    with tile.TileContext(nc) as tc:
        with (
            tc.tile_pool(name="consts", bufs=1) as consts,
            tc.tile_pool(name="pf", bufs=2) as pfpool,
            tc.tile_pool(name="ofp", bufs=2) as ofpool,
            tc.tile_pool(name="work", bufs=2) as work,
            tc.tile_pool(name="ps_scores", bufs=2, space="PSUM") as ps_scores,
            tc.tile_pool(name="ps_tp", bufs=2, space="PSUM") as ps_tp,
            tc.tile_pool(name="ps_ctx", bufs=1, space="PSUM") as ps_ctx,
            tc.tile_pool(name="ps_out", bufs=1, space="PSUM") as ps_out,
        ):
            # ---- constants (loaded once) ----
            peT_sb = consts.tile([128, pe_terms, HT, T], mm_dt)
            nc.sync.dma_start(peT_sb, peT.rearrange("e (ht p) t -> p e ht t", p=128))
            wgT_sb = consts.tile([128, HT, C], mm_dt)
            nc.sync.dma_start(wgT_sb, wgT.rearrange("(ht p) c -> p ht c", p=128))
            bg_sb = consts.tile([C, 1], F32)
            nc.sync.dma_start(bg_sb, bg)
            ident = consts.tile([128, 128], mm_dt)
            make_identity(nc, ident)

            for _rep in range(repeats):
              for b in range(BPC):
                # ---- load inputs for this batch ----
                pf_sb = pfpool.tile([128, HT, L], mm_dt, tag="pf")
                nc.sync.dma_start(pf_sb, pfT[b].rearrange("(ht p) l -> p ht l", p=128))
                of_sb = ofpool.tile([128, LT, H], mm_dt, tag="of")
                nc.sync.dma_start(of_sb, of[b].rearrange("(lt p) h -> p lt h", p=128))

                # ---- matmul 1: scores[T, L] += peT[e,h]^T @ pfT[h] ----
                sc_ps = ps_scores.tile([T, L], F32, tag="scores")
                for lh in range(L // 512):
                    first, last = True, False
                    for e in range(pe_terms):
                        for ht in range(HT):
                            last = e == pe_terms - 1 and ht == HT - 1
                            nc.tensor.matmul(
                                sc_ps[:, lh * 512 : (lh + 1) * 512],
                                lhsT=peT_sb[:, e, ht, :],
                                rhs=pf_sb[:, ht, lh * 512 : (lh + 1) * 512],
                                start=first,
                                stop=last,
                            )
                            first = False

                # ---- softmax over free dim (L) ----
                negm = work.tile([T, 1], F32, tag="negm")
                nc.vector.tensor_reduce(negm, sc_ps[:], axis=AX.X, op=OP.max, negate=True)
                p_sb = work.tile([T, L], mm_dt, tag="p")
                ssum = work.tile([T, 1], F32, tag="ssum")
                nc.scalar.activation(p_sb, sc_ps[:], AF.Exp, bias=negm, scale=1.0, accum_out=ssum)
                rinv = work.tile([T, 1], F32, tag="rinv")
                nc.vector.reciprocal(rinv, ssum)

                # ---- transpose P: [T, L] -> pT [128, LT, T] (PE transpose) ----
                pT_sb = work.tile([128, LT, T], mm_dt, tag="pT")
                for lt in range(LT):
                    tp_ps = ps_tp.tile([128, T], mm_dt, tag="tp")
                    nc.tensor.transpose(tp_ps, p_sb[:, lt * 128 : (lt + 1) * 128], ident[:T, :T])
                    if lt % 2 == 0:
                        nc.vector.tensor_copy(pT_sb[:, lt, :], tp_ps)
                    else:
                        nc.scalar.copy(pT_sb[:, lt, :], tp_ps)

                # ---- matmul 2: ctx[T, H] += pT[l]^T @ of[l] ----
                ctx_ps = ps_ctx.tile([T, H], F32, tag="ctx")
                for lt in range(LT):
                    nc.tensor.matmul(
                        ctx_ps,
                        lhsT=pT_sb[:, lt, :],
                        rhs=of_sb[:, lt, :],
                        start=(lt == 0),
                        stop=(lt == LT - 1),
                    )
                # scale by 1/sum during PSUM->SBUF copy (+ cast)
                ctx_sb = work.tile([T, H], mm_dt, tag="ctx_sb")
                nc.vector.tensor_scalar_mul(ctx_sb, ctx_ps[:], rinv)

                # ---- transpose ctx: [T, H] -> cT [128, HT, T] ----
                cT_sb = work.tile([128, HT, T], mm_dt, tag="cT")
                for ht in range(HT):
                    tp_ps = ps_tp.tile([128, T], mm_dt, tag="tp")
                    nc.tensor.transpose(tp_ps, ctx_sb[:, ht * 128 : (ht + 1) * 128], ident[:T, :T])
                    if ht % 2 == 0:
                        nc.vector.tensor_copy(cT_sb[:, ht, :], tp_ps)
                    else:
                        nc.scalar.copy(cT_sb[:, ht, :], tp_ps)

                # ---- matmul 3: outT[C, T] += wgT[h]^T @ cT[h] ----
                o_ps = ps_out.tile([C, T], F32, tag="out")
                for ht in range(HT):
                    nc.tensor.matmul(
                        o_ps,
                        lhsT=wgT_sb[:, ht, :],
                        rhs=cT_sb[:, ht, :],
                        start=(ht == 0),
                        stop=(ht == HT - 1),
                    )
                out_sb = work.tile([C, T], F32, tag="out_sb")
                nc.vector.tensor_scalar_add(out_sb, o_ps[:], bg_sb)
                nc.sync.dma_start(outT[b], out_sb)

    nc.compile()
    return nc


_NC = None


def _get_nc():
    global _NC
    if _NC is None:
        _NC = build_nc()
    return _NC


def make_in_maps(position_fmap, origin_fmap, pos_emb, W_gen, b_gen, np_dt=NP_DT, pe_terms=PE_TERMS):
    """Host-side sharding + layout prep. Returns list of per-core input dicts."""
    pf = np.asarray(position_fmap, dtype=np.float32)
    of = np.asarray(origin_fmap, dtype=np.float32)
    pe = np.asarray(pos_emb, dtype=np.float32)
    wg = np.asarray(W_gen, dtype=np.float32)
    bgv = np.asarray(b_gen, dtype=np.float32)

    # [B, L, H] -> [B, H, L], contiguous
    pfT = np.ascontiguousarray(pf.transpose(0, 2, 1)).astype(np_dt)
    of_c = np.ascontiguousarray(of).astype(np_dt)

    peT_f32 = np.ascontiguousarray(pe.T)  # [H, T]
    terms = []
    resid = peT_f32
    for _ in range(pe_terms):
        t = resid.astype(np_dt)
        terms.append(t)
        resid = resid - t.astype(np.float32)
    peT = np.stack(terms, axis=0)  # [pe_terms, H, T]

    wgT = np.ascontiguousarray(wg.T).astype(np_dt)
    bg2 = np.ascontiguousarray(bgv.reshape(C, 1)).astype(np.float32)

    in_maps = []
    for i in range(NCORES):
        sl = slice(i * BPC, (i + 1) * BPC)
        in_maps.append(
            {
                "pfT": pfT[sl],
                "of": of_c[sl],
                "peT": peT,
                "wgT": wgT,
                "bg": bg2,
            }
        )
    return in_maps


def kernel(position_fmap, origin_fmap, pos_emb, W_gen, b_gen):
    nc = _get_nc()
    in_maps = make_in_maps(position_fmap, origin_fmap, pos_emb, W_gen, b_gen)
    res = run_bass_kernel_spmd(nc, in_maps, core_ids=list(range(NCORES)))
    outs = [r["outT"] for r in res.results]  # each [BPC, C, T]
    out = np.concatenate(outs, axis=0)  # [B, C, T]
    return np.ascontiguousarray(out.transpose(0, 2, 1)).astype(np.float32)
